# revision 17
# baseline (speedup 1.0000x reference)
"""Trainium2 Bass kernel for brute-force kNN — v3 (per-tile entry summaries).

Device (per core, candidates sharded along N, queries replicated):
  - cT [128, W] bf16: partition rows 0:64 = dims of shard[:W] (half 0),
    rows 64:128 = dims of shard[W:] padded (half 1)  [same as baseline].
  - Per DMA tile (2048 cols) x query-group g x half h x sub-col c2:
    PSUM tile [128, 1024] fp32 scores via 2 matmuls (N=512, row-paired
    h0/h1 for PE row-group concurrency).
  - Each PSUM tile is summarized by ONE instruction into one entry per
    query:
      V-route: DVE reduce_max  -> ent_v[:, col]   (exact tile max)
      A-route: ACT exp(s-BIAS) written in place over the PSUM tile,
               accum_out -> ent_a[:, col]  (sum of exp: ln(v)+BIAS is an
               exact upper bound on the tile max, and a tight one)
  - ent tables [2, 128, 124] DMA'd out (128 KB total per core).

Host: adaptive exact rescore. Per query, sort entry upper bounds desc;
score entries (<=1024 candidates each, exact fp32) until the 10th-best
scored candidate exceeds the next entry bound (+slack for bf16/spline
error). Provably returns the exact fp32 top-k.
"""

from contextlib import ExitStack, nullcontext

import ml_dtypes
import numpy as np

import concourse.bass as bass
import concourse.mybir as mybir
import concourse.tile as tile
from concourse.bass_utils import run_bass_kernel_spmd

f32 = mybir.dt.float32
bf16 = mybir.dt.bfloat16

B = 256
D = 64
N = 1_000_000
NCORES = 8
NSHARD = N // NCORES          # 125000
W = 63488                     # half-shard width (31 * 2048)
F_DMA = 2048
N_DMA_TILES = W // F_DMA      # 31
FT = 1024                     # summary tile width
EXP_BIAS = 35.0               # entry_a = sum(exp(s - EXP_BIAS))

# V-share: measured-optimal routing split; 62/62 strict V/A alternation
# (sweeps: 0.46->201k, 0.495->158.6k, 0.515->173k ns)
FV = 0.495

# A-route writes exp() back over the PSUM tile itself instead of to an
# SBUF scratch (ScalarE is closer to PSUM; frees the SBUF write path).
# Measured: 1496 ns/op vs 1745 with SBUF scratch.
ACT_INPLACE = True

_MAX_WAITS = 1


def _split_excess_waits(nc):
    """Walrus rejects instructions with >1 sem-wait; move extras onto
    same-engine NoOps immediately before the instruction."""
    n_nops = 0
    for f in nc.m.functions:
        for bb in f.blocks:
            new_insts = []
            dirty = False
            for ins in bb.instructions:
                si = ins.sync_info
                if (
                    si is not None
                    and si.on_wait is not None
                    and len(si.on_wait) > _MAX_WAITS
                ):
                    waits = list(si.on_wait)
                    keep = waits[:_MAX_WAITS]
                    rest = waits[_MAX_WAITS:]
                    for j in range(0, len(rest), _MAX_WAITS):
                        nop = mybir.InstNoOp(name=f"I-waitsplit-{n_nops}")
                        n_nops += 1
                        nop.engine = ins.engine
                        nop.sync_info = mybir.SyncInfo(
                            on_wait=rest[j : j + _MAX_WAITS], on_update=[]
                        )
                        new_insts.append(nop)
                    ins.sync_info = mybir.SyncInfo(
                        on_wait=keep, on_update=list(si.on_update or [])
                    )
                    dirty = True
                new_insts.append(ins)
            if dirty:
                bb.instructions = new_insts
    return n_nops


def tile_schedule(fv: float = FV):
    """Per query-group: list of (ti, h, c2, route, col).

    route 'V' -> DVE reduce entry, col into ent_v; 'A' -> ACT exp entry.
    At fv=0.495 this yields strict V,A,V,A alternation (62/62), the
    measured-optimal pool ping-pong (A-first, ratio shifts, and skipping
    the mostly-padding tail tile all measured slower).
    Same schedule for every g and every core. Host depends on this.
    """
    sched = []
    nv = na = 0
    for ti in range(N_DMA_TILES):
        for c2 in range(2):
            for h in range(2):
                if nv < fv * (nv + na + 1):
                    sched.append((ti, h, c2, "V", nv))
                    nv += 1
                else:
                    sched.append((ti, h, c2, "A", na))
                    na += 1
    return sched, nv, na


SCHED, NV, NA = tile_schedule()
NENT = NV + NA  # 124


def _build_nc(repeat: int = 1, loop_repeat: int = 0, cpool_bufs: int = 3,
              staggered: bool = True, ppv_bufs: int = 2, ppa_bufs: int = 2):
    nc = bass.Bass()
    qT = nc.dram_tensor("qT", [128, B], bf16, kind="ExternalInput")
    cT = nc.dram_tensor("cT", [128, W], bf16, kind="ExternalInput")
    bm = nc.dram_tensor("bm", [2, 128, NENT], f32, kind="ExternalOutput")

    with tile.TileContext(nc) as tc, ExitStack() as ctx:
        qpool = ctx.enter_context(tc.tile_pool(name="qpool", bufs=1))
        cpool = ctx.enter_context(tc.tile_pool(name="cpool", bufs=cpool_bufs))
        ppv = ctx.enter_context(
            tc.tile_pool(name="ppv", bufs=ppv_bufs, space="PSUM")
        )
        ppa = ctx.enter_context(
            tc.tile_pool(name="ppa", bufs=ppa_bufs, space="PSUM")
        )
        sbp = ctx.enter_context(tc.tile_pool(name="sbp", bufs=1))

        qt = qpool.tile([128, B], bf16)
        nc.sync.dma_start(out=qt[:], in_=qT[:])
        biasT = sbp.tile([128, 1], f32, name="biasT", tag="biasT")
        nc.gpsimd.memset(biasT[:], -EXP_BIAS)
        ent = [
            sbp.tile([128, NENT], f32, name=f"ent{g}", tag=f"ent{g}")
            for g in range(2)
        ]
        scr = [
            sbp.tile([128, FT], bf16, name=f"scr{i}", tag=f"scr{i}")
            for i in range(2)
        ]

        # group schedule by (ti, c2) so the two halves' matmuls interleave
        # (row groups 0:64 / 64:128 run concurrently on the PE)
        by_tile = {}
        for (ti, h, c2, route, col) in SCHED:
            by_tile[(ti, c2, h)] = (route, col)

        def body_ctx():
            if loop_repeat > 0:
                return tc.For_i(
                    0, loop_repeat, 1,
                    hint_engines=(
                        mybir.EngineType.PE,
                        mybir.EngineType.DVE,
                        mybir.EngineType.SP,
                        mybir.EngineType.Activation,
                    ),
                    staggered_reset=staggered,
                )
            return nullcontext()

        n_scr = 0
        with body_ctx():
          for _rep in range(repeat):
            for ti in range(N_DMA_TILES):
                ct = cpool.tile([128, F_DMA], bf16)
                nc.sync.dma_start(
                    out=ct[:], in_=cT[:, ti * F_DMA : (ti + 1) * F_DMA]
                )
                for g in range(2):
                    for c2 in range(2):
                        tiles = {}
                        for h in range(2):
                            if (ti, c2, h) not in by_tile:
                                continue  # all-padding tile, skipped
                            route, col = by_tile[(ti, c2, h)]
                            pool = ppv if route == "V" else ppa
                            tiles[h] = (
                                pool.tile(
                                    [128, FT], f32,
                                    name=f"ps{route}", tag=f"ps{route}",
                                ),
                                route,
                                col,
                            )
                        # matmuls: interleave h0/h1 for row-group overlap
                        for sub in range(2):
                            for h in tiles:
                                pt = tiles[h][0]
                                nc.tensor.matmul(
                                    out=pt[:, 512 * sub : 512 * (sub + 1)],
                                    lhsT=qt[
                                        64 * h : 64 * (h + 1),
                                        128 * g : 128 * (g + 1),
                                    ],
                                    rhs=ct[
                                        64 * h : 64 * (h + 1),
                                        c2 * FT
                                        + 512 * sub : c2 * FT
                                        + 512 * (sub + 1),
                                    ],
                                    start=True,
                                    stop=True,
                                )
                        for h in tiles:
                            pt, route, col = tiles[h]
                            if route == "V":
                                nc.vector.reduce_max(
                                    out=ent[g][:, col : col + 1],
                                    in_=pt[:].rearrange(
                                        "p (a n) -> p a n", a=1
                                    ),
                                    axis=mybir.AxisListType.X,
                                )
                            else:
                                nc.scalar.activation(
                                    out=pt[:] if ACT_INPLACE else scr[n_scr % 2][:],
                                    in_=pt[:],
                                    func=mybir.ActivationFunctionType.Exp,
                                    bias=biasT[:],
                                    scale=1.0,
                                    accum_out=ent[g][:, NV + col : NV + col + 1],
                                )
                                n_scr += 1
            if loop_repeat == 0:
                for g in range(2):
                    nc.sync.dma_start(out=bm[g], in_=ent[g][:])
        if loop_repeat > 0:
            for g in range(2):
                nc.sync.dma_start(out=bm[g], in_=ent[g][:])
    _split_excess_waits(nc)
    nc.finalize()
    return nc


_NC_CACHE: dict[tuple, object] = {}


def get_nc(repeat: int = 1):
    key = (repeat,)
    if key not in _NC_CACHE:
        _NC_CACHE[key] = _build_nc(repeat)
    return _NC_CACHE[key]


def _prep_inputs(queries: np.ndarray, candidates: np.ndarray):
    q = np.asarray(queries, dtype=np.float32)
    c = np.asarray(candidates, dtype=np.float32)
    qT = np.ascontiguousarray(q.T)  # [64, 256]
    qT2 = np.concatenate([qT, qT], axis=0).astype(ml_dtypes.bfloat16)
    in_maps = []
    for core in range(NCORES):
        shard = c[core * NSHARD : (core + 1) * NSHARD]
        half_a = shard[:W]
        half_b = shard[W:]
        cT2 = np.zeros((128, W), dtype=ml_dtypes.bfloat16)
        cT2[:D, :] = half_a.T.astype(ml_dtypes.bfloat16)
        cT2[D:, : half_b.shape[0]] = half_b.T.astype(ml_dtypes.bfloat16)
        in_maps.append({"qT": qT2, "cT": cT2})
    return in_maps


# slack on entry upper bounds: covers bf16-input score error vs exact
# fp32 host rescore, plus ACT exp spline error on the A-entries.
DELTA_V = 0.5
DELTA_A = 0.7


def _host_finish(bm_all, queries, candidates, ids, k):
    """bm_all: [NCORES, 2, 128, NENT] f32 -> exact top-k via adaptive
    bound-ordered rescore."""
    q = np.asarray(queries, dtype=np.float32)
    c = np.asarray(candidates, dtype=np.float32)
    ids = np.asarray(ids)
    k = int(k)

    # entry metadata (same for every core/group): candidate start offsets
    starts = np.empty(NENT, dtype=np.int64)   # shard-local start
    for (ti, h, c2, route, col) in SCHED:
        idx = col if route == "V" else NV + col
        starts[idx] = h * W + ti * F_DMA + c2 * FT

    # upper bounds [B, NCORES, NENT]
    bmq = bm_all.transpose(1, 2, 0, 3).reshape(B, NCORES, NENT)
    bounds = np.empty_like(bmq)
    bounds[:, :, :NV] = bmq[:, :, :NV] + DELTA_V
    with np.errstate(divide="ignore"):
        bounds[:, :, NV:] = (
            np.log(np.maximum(bmq[:, :, NV:], 1e-38)) + EXP_BIAS + DELTA_A
        )

    top_scores = np.empty((B, k), dtype=np.float32)
    top_idx = np.empty((B, k), dtype=np.int32)

    flat_bounds = bounds.reshape(B, NCORES * NENT)
    order_all = np.argsort(-flat_bounds, axis=1)

    for qq in range(B):
        qv = q[qq]
        order = order_all[qq]
        fb = flat_bounds[qq]
        scores_acc = []
        idx_acc = []
        kth = -np.inf
        for oi, e in enumerate(order):
            if fb[e] < kth and len(scores_acc) >= 1:
                break
            core, ent_i = divmod(e, NENT)
            s0 = core * NSHARD + starts[ent_i]
            s1 = min(s0 + FT, (core + 1) * NSHARD)
            if s1 <= s0:
                continue
            block = c[s0:s1]
            sc = block @ qv
            scores_acc.append(sc.astype(np.float32))
            idx_acc.append(np.arange(s0, s1, dtype=np.int64))
            if len(scores_acc) >= 8 or oi + 1 == len(order):
                allsc = np.concatenate(scores_acc)
                if len(allsc) >= k:
                    kth = np.partition(allsc, -k)[-k]
        allsc = np.concatenate(scores_acc)
        allix = np.concatenate(idx_acc)
        m = min(len(allsc) - 1, 4 * k)
        sel = np.argpartition(-allsc, m)[: m + 1]
        o2 = np.lexsort((allix[sel], -allsc[sel]))
        out_s, out_i = [], []
        for o in o2:
            out_s.append(allsc[sel[o]])
            out_i.append(allix[sel[o]])
            if len(out_s) == k:
                break
        top_scores[qq] = out_s
        top_idx[qq] = ids[np.asarray(out_i, dtype=np.int64)]
    return top_scores, top_idx


def kernel(queries, candidates, ids, k):
    k = int(k)
    in_maps = _prep_inputs(queries, candidates)
    nc = get_nc(repeat=1)
    res = run_bass_kernel_spmd(nc, in_maps, core_ids=list(range(NCORES)))
    bm_all = np.stack([res.results[c]["bm"] for c in range(NCORES)])
    return _host_finish(
        bm_all,
        np.asarray(queries, np.float32),
        np.asarray(candidates, np.float32),
        np.asarray(ids),
        k,
    )


# revision 21
# speedup vs baseline: 1.0258x; 1.0258x over previous
"""Trainium2 Bass kernel for brute-force kNN — v3 (per-tile entry summaries).

Device (per core, candidates sharded along N, queries replicated):
  - cT [128, W] bf16: partition rows 0:64 = dims of shard[:W] (half 0),
    rows 64:128 = dims of shard[W:] padded (half 1)  [same as baseline].
  - Per DMA tile (2048 cols) x query-group g x half h x sub-col c2:
    PSUM tile [128, 1024] fp32 scores via 2 matmuls (N=512, row-paired
    h0/h1 for PE row-group concurrency).
  - Each PSUM tile is summarized by ONE instruction into one entry per
    query:
      V-route: DVE reduce_max  -> ent_v[:, col]   (exact tile max)
      A-route: ACT exp(s-BIAS) written in place over the PSUM tile,
               accum_out -> ent_a[:, col]  (sum of exp: ln(v)+BIAS is an
               exact upper bound on the tile max, and a tight one)
  - ent tables [2, 128, 124] DMA'd out (128 KB total per core).

Host: adaptive exact rescore. Per query, sort entry upper bounds desc;
score entries (<=1024 candidates each, exact fp32) until the 10th-best
scored candidate exceeds the next entry bound (+slack for bf16/spline
error). Provably returns the exact fp32 top-k.
"""

from contextlib import ExitStack, nullcontext

import ml_dtypes
import numpy as np

import concourse.bass as bass
import concourse.mybir as mybir
import concourse.tile as tile
from concourse.bass_utils import run_bass_kernel_spmd

f32 = mybir.dt.float32
bf16 = mybir.dt.bfloat16

B = 256
D = 64
N = 1_000_000
NCORES = 8
NSHARD = N // NCORES          # 125000
W = 63488                     # half-shard width (31 * 2048)
F_DMA = 2048
N_DMA_TILES = W // F_DMA      # 31
FT = 1024                     # summary tile width
EXP_BIAS = 35.0               # entry_a = sum(exp(s - EXP_BIAS))

# V-share: measured-optimal routing split; 62/62 strict V/A alternation
# (sweeps: 0.46->201k, 0.495->158.6k, 0.515->173k ns)
FV = 0.495

# A-route writes exp() back over the PSUM tile itself instead of to an
# SBUF scratch (ScalarE is closer to PSUM; frees the SBUF write path).
# Measured: 1496 ns/op vs 1745 with SBUF scratch.
ACT_INPLACE = True

_MAX_WAITS = 1


def _split_excess_waits(nc):
    """Walrus rejects instructions with >1 sem-wait; move extras onto
    same-engine NoOps immediately before the instruction."""
    n_nops = 0
    for f in nc.m.functions:
        for bb in f.blocks:
            new_insts = []
            dirty = False
            for ins in bb.instructions:
                si = ins.sync_info
                if (
                    si is not None
                    and si.on_wait is not None
                    and len(si.on_wait) > _MAX_WAITS
                ):
                    waits = list(si.on_wait)
                    keep = waits[:_MAX_WAITS]
                    rest = waits[_MAX_WAITS:]
                    for j in range(0, len(rest), _MAX_WAITS):
                        nop = mybir.InstNoOp(name=f"I-waitsplit-{n_nops}")
                        n_nops += 1
                        nop.engine = ins.engine
                        nop.sync_info = mybir.SyncInfo(
                            on_wait=rest[j : j + _MAX_WAITS], on_update=[]
                        )
                        new_insts.append(nop)
                    ins.sync_info = mybir.SyncInfo(
                        on_wait=keep, on_update=list(si.on_update or [])
                    )
                    dirty = True
                new_insts.append(ins)
            if dirty:
                bb.instructions = new_insts
    return n_nops


def tile_schedule(fv: float = FV):
    """Per query-group: list of (ti, h, c2, route, col).

    route 'V' -> DVE reduce entry, col into ent_v; 'A' -> ACT exp entry.
    At fv=0.495 this yields strict V,A,V,A alternation (62/62), the
    measured-optimal pool ping-pong (A-first, ratio shifts, and skipping
    the mostly-padding tail tile all measured slower).
    Same schedule for every g and every core. Host depends on this.
    """
    sched = []
    nv = na = 0
    for ti in range(N_DMA_TILES):
        for c2 in range(2):
            for h in range(2):
                if nv < fv * (nv + na + 1):
                    sched.append((ti, h, c2, "V", nv))
                    nv += 1
                else:
                    sched.append((ti, h, c2, "A", na))
                    na += 1
    return sched, nv, na


SCHED, NV, NA = tile_schedule()
NENT = NV + NA  # 124


def _build_nc(repeat: int = 1, loop_repeat: int = 0, cpool_bufs: int = 3,
              staggered: bool = True, ppv_bufs: int = 2, ppa_bufs: int = 2):
    nc = bass.Bass()
    qT = nc.dram_tensor("qT", [128, B], bf16, kind="ExternalInput")
    cT = nc.dram_tensor("cT", [128, W], bf16, kind="ExternalInput")
    bm = nc.dram_tensor("bm", [2, 128, NENT], f32, kind="ExternalOutput")

    with tile.TileContext(nc) as tc, ExitStack() as ctx:
        qpool = ctx.enter_context(tc.tile_pool(name="qpool", bufs=1))
        cpool = ctx.enter_context(tc.tile_pool(name="cpool", bufs=cpool_bufs))
        ppv = ctx.enter_context(
            tc.tile_pool(name="ppv", bufs=ppv_bufs, space="PSUM")
        )
        ppa = ctx.enter_context(
            tc.tile_pool(name="ppa", bufs=ppa_bufs, space="PSUM")
        )
        sbp = ctx.enter_context(tc.tile_pool(name="sbp", bufs=1))

        qt = qpool.tile([128, B], bf16)
        nc.sync.dma_start(out=qt[:], in_=qT[:])
        biasT = sbp.tile([128, 1], f32, name="biasT", tag="biasT")
        nc.gpsimd.memset(biasT[:], -EXP_BIAS)

        ent = [
            sbp.tile([128, NENT], f32, name=f"ent{g}", tag=f"ent{g}")
            for g in range(2)
        ]
        scr = [
            sbp.tile([128, FT], bf16, name=f"scr{i}", tag=f"scr{i}")
            for i in range(2)
        ]

        # group schedule by (ti, c2) so the two halves' matmuls interleave
        # (row groups 0:64 / 64:128 run concurrently on the PE)
        by_tile = {}
        for (ti, h, c2, route, col) in SCHED:
            by_tile[(ti, c2, h)] = (route, col)

        def body_ctx():
            if loop_repeat > 0:
                return tc.For_i(
                    0, loop_repeat, 1,
                    hint_engines=(
                        mybir.EngineType.PE,
                        mybir.EngineType.DVE,
                        mybir.EngineType.SP,
                        mybir.EngineType.Activation,
                    ),
                    staggered_reset=staggered,
                )
            return nullcontext()

        n_scr = 0
        with body_ctx():
          for _rep in range(repeat):
            for ti in range(N_DMA_TILES):
                ct = cpool.tile([128, F_DMA], bf16)
                nc.sync.dma_start(
                    out=ct[:], in_=cT[:, ti * F_DMA : (ti + 1) * F_DMA]
                )
                for g in range(2):
                    for c2 in range(2):
                        tiles = {}
                        for h in range(2):
                            if (ti, c2, h) not in by_tile:
                                continue  # all-padding tile, skipped
                            route, col = by_tile[(ti, c2, h)]
                            pool = ppv if route == "V" else ppa
                            tiles[h] = (
                                pool.tile(
                                    [128, FT], f32,
                                    name=f"ps{route}", tag=f"ps{route}",
                                ),
                                route,
                                col,
                            )
                        # matmuls: interleave h0/h1 for row-group overlap
                        for sub in range(2):
                            for h in tiles:
                                pt = tiles[h][0]
                                nc.tensor.matmul(
                                    out=pt[:, 512 * sub : 512 * (sub + 1)],
                                    lhsT=qt[
                                        64 * h : 64 * (h + 1),
                                        128 * g : 128 * (g + 1),
                                    ],
                                    rhs=ct[
                                        64 * h : 64 * (h + 1),
                                        c2 * FT
                                        + 512 * sub : c2 * FT
                                        + 512 * (sub + 1),
                                    ],
                                    start=True,
                                    stop=True,
                                )
                        for h in tiles:
                            pt, route, col = tiles[h]
                            if route == "V":
                                nc.vector.reduce_max(
                                    out=ent[g][:, col : col + 1],
                                    in_=pt[:].rearrange(
                                        "p (a n) -> p a n", a=1
                                    ),
                                    axis=mybir.AxisListType.X,
                                )
                            else:
                                nc.scalar.activation(
                                    out=pt[:] if ACT_INPLACE else scr[n_scr % 2][:],
                                    in_=pt[:],
                                    func=mybir.ActivationFunctionType.Exp,
                                    bias=biasT[:],
                                    scale=1.0,
                                    accum_out=ent[g][:, NV + col : NV + col + 1],
                                )
                                n_scr += 1
            if loop_repeat == 0:
                for g in range(2):
                    nc.sync.dma_start(out=bm[g], in_=ent[g][:])
        if loop_repeat > 0:
            for g in range(2):
                nc.sync.dma_start(out=bm[g], in_=ent[g][:])
    _split_excess_waits(nc)
    nc.finalize()
    return nc


_NC_CACHE: dict[tuple, object] = {}


def get_nc(repeat: int = 1):
    key = (repeat,)
    if key not in _NC_CACHE:
        _NC_CACHE[key] = _build_nc(repeat)
    return _NC_CACHE[key]


def _prep_inputs(queries: np.ndarray, candidates: np.ndarray):
    q = np.asarray(queries, dtype=np.float32)
    c = np.asarray(candidates, dtype=np.float32)
    qT = np.ascontiguousarray(q.T)  # [64, 256]
    qT2 = np.concatenate([qT, qT], axis=0).astype(ml_dtypes.bfloat16)
    in_maps = []
    for core in range(NCORES):
        shard = c[core * NSHARD : (core + 1) * NSHARD]
        half_a = shard[:W]
        half_b = shard[W:]
        cT2 = np.zeros((128, W), dtype=ml_dtypes.bfloat16)
        cT2[:D, :] = half_a.T.astype(ml_dtypes.bfloat16)
        cT2[D:, : half_b.shape[0]] = half_b.T.astype(ml_dtypes.bfloat16)
        in_maps.append({"qT": qT2, "cT": cT2})
    return in_maps


# slack on entry upper bounds: covers bf16-input score error vs exact
# fp32 host rescore, plus ACT exp spline error on the A-entries.
DELTA_V = 0.5
DELTA_A = 0.7


def _host_finish(bm_all, queries, candidates, ids, k):
    """bm_all: [NCORES, 2, 128, NENT] f32 -> exact top-k via adaptive
    bound-ordered rescore."""
    q = np.asarray(queries, dtype=np.float32)
    c = np.asarray(candidates, dtype=np.float32)
    ids = np.asarray(ids)
    k = int(k)

    # entry metadata (same for every core/group): candidate start offsets
    starts = np.empty(NENT, dtype=np.int64)   # shard-local start
    for (ti, h, c2, route, col) in SCHED:
        idx = col if route == "V" else NV + col
        starts[idx] = h * W + ti * F_DMA + c2 * FT

    # upper bounds [B, NCORES, NENT]
    bmq = bm_all.transpose(1, 2, 0, 3).reshape(B, NCORES, NENT)
    bounds = np.empty_like(bmq)
    bounds[:, :, :NV] = bmq[:, :, :NV] + DELTA_V
    with np.errstate(divide="ignore"):
        bounds[:, :, NV:] = (
            np.log(np.maximum(bmq[:, :, NV:], 1e-38)) + EXP_BIAS + DELTA_A
        )

    top_scores = np.empty((B, k), dtype=np.float32)
    top_idx = np.empty((B, k), dtype=np.int32)

    flat_bounds = bounds.reshape(B, NCORES * NENT)
    order_all = np.argsort(-flat_bounds, axis=1)

    for qq in range(B):
        qv = q[qq]
        order = order_all[qq]
        fb = flat_bounds[qq]
        scores_acc = []
        idx_acc = []
        kth = -np.inf
        for oi, e in enumerate(order):
            if fb[e] < kth and len(scores_acc) >= 1:
                break
            core, ent_i = divmod(e, NENT)
            s0 = core * NSHARD + starts[ent_i]
            s1 = min(s0 + FT, (core + 1) * NSHARD)
            if s1 <= s0:
                continue
            block = c[s0:s1]
            sc = block @ qv
            scores_acc.append(sc.astype(np.float32))
            idx_acc.append(np.arange(s0, s1, dtype=np.int64))
            if len(scores_acc) >= 8 or oi + 1 == len(order):
                allsc = np.concatenate(scores_acc)
                if len(allsc) >= k:
                    kth = np.partition(allsc, -k)[-k]
        allsc = np.concatenate(scores_acc)
        allix = np.concatenate(idx_acc)
        m = min(len(allsc) - 1, 4 * k)
        sel = np.argpartition(-allsc, m)[: m + 1]
        o2 = np.lexsort((allix[sel], -allsc[sel]))
        out_s, out_i = [], []
        for o in o2:
            out_s.append(allsc[sel[o]])
            out_i.append(allix[sel[o]])
            if len(out_s) == k:
                break
        top_scores[qq] = out_s
        top_idx[qq] = ids[np.asarray(out_i, dtype=np.int64)]
    return top_scores, top_idx


def kernel(queries, candidates, ids, k):
    k = int(k)
    in_maps = _prep_inputs(queries, candidates)
    nc = get_nc(repeat=1)
    res = run_bass_kernel_spmd(nc, in_maps, core_ids=list(range(NCORES)))
    bm_all = np.stack([res.results[c]["bm"] for c in range(NCORES)])
    return _host_finish(
        bm_all,
        np.asarray(queries, np.float32),
        np.asarray(candidates, np.float32),
        np.asarray(ids),
        k,
    )
